# revision 1
# baseline (speedup 1.0000x reference)
import math
from contextlib import ExitStack

import numpy as np

from concourse import bacc, bass, mybir, tile
from concourse.bass_utils import run_bass_kernel_spmd

dt = mybir.dt
AF = mybir.ActivationFunctionType
ALU = mybir.AluOpType

B = 4
L = 256
D_PAIR = 288
D_HALF = 144
MAX_LEN = 260
VOCAB = 21
IH = 128
JG = 8
N_CORES = 8


def _pos_enc_table() -> np.ndarray:
    idx = np.arange(0, D_HALF, 2, dtype=np.float32)
    t = (np.float32(math.log(10000.0)) * idx) / np.float32(D_HALF)
    denom = np.exp(t, dtype=np.float32)
    pos = np.arange(MAX_LEN, dtype=np.float32)[:, None]
    pe = np.zeros((MAX_LEN, D_HALF), dtype=np.float32)
    pe[:, 0::2] = np.sin(pos / denom, dtype=np.float32)
    pe[:, 1::2] = np.cos(pos / denom, dtype=np.float32)
    return pe


def _bcast(ap_src, nparts: int):
    return bass.AP(
        tensor=ap_src.tensor, offset=ap_src.offset, ap=[[0, nparts], *ap_src.ap]
    )


def build(stage: str = "full", repeat: int = 1, variant: str = "") -> bass.Bass:
    nc = bacc.Bacc("TRN2", target_bir_lowering=False)

    seqb_d = nc.dram_tensor("seqb", [L], dt.int32, kind="ExternalInput")
    seqi_d = nc.dram_tensor("seqi", [IH], dt.int32, kind="ExternalInput")
    aab_d = nc.dram_tensor("aab", [L], dt.int32, kind="ExternalInput")
    aai_d = nc.dram_tensor("aai", [IH], dt.int32, kind="ExternalInput")
    emb_d = nc.dram_tensor("emb", [VOCAB, D_HALF], dt.float32, kind="ExternalInput")
    wp_d = nc.dram_tensor("wp", [D_PAIR + 1, D_PAIR], dt.float32, kind="ExternalInput")
    bp_d = nc.dram_tensor("bp", [D_PAIR], dt.float32, kind="ExternalInput")
    out_d = nc.dram_tensor("out", [IH, L, D_PAIR], dt.float32, kind="ExternalOutput")

    pos_np = _pos_enc_table()
    posL_np = np.zeros((128, 3 * D_PAIR), dtype=np.float32)
    posR_np = np.zeros((128, 3 * D_PAIR), dtype=np.float32)
    for c in range(3):
        rows = 128 if c < 2 else MAX_LEN - 256
        chunk = pos_np[c * 128 : c * 128 + rows, :]
        posL_np[0:rows, c * D_PAIR : c * D_PAIR + D_HALF] = chunk
        posR_np[0:rows, c * D_PAIR + D_HALF : (c + 1) * D_PAIR] = chunk
    posL_d = nc.inline_tensor(posL_np, "posL_c")
    posR_d = nc.inline_tensor(posR_np, "posR_c")
    iota_np = (
        np.arange(128, dtype=np.float32)[:, None]
        + 128.0 * np.arange(3, dtype=np.float32)[None, :]
    ).astype(np.float32)
    iota_d = nc.inline_tensor(iota_np, "iota")

    with tile.TileContext(nc) as tc, ExitStack() as ctx:
        persist = ctx.enter_context(tc.tile_pool(name="persist", bufs=1))

        flat_t = persist.tile([2, L * D_PAIR], dt.bfloat16, tag="flat")
        ones_t = persist.tile([2, IH], dt.bfloat16, tag="ones")
        cp_t = persist.tile([IH, D_PAIR], dt.float32, tag="cpt")
        wsep_t = persist.tile([IH, D_PAIR], dt.float32, tag="wsep")
        sep_t = persist.tile([IH, L], dt.float32, tag="sept")

        nc.vector.memset(ones_t, 1.0)

        with ExitStack() as pre:
            scr = pre.enter_context(tc.tile_pool(name="scr", bufs=1))
            psc = pre.enter_context(tc.tile_pool(name="psc", bufs=1, space="PSUM"))

            iota_t = scr.tile([128, 3], dt.float32, tag="iota")
            nc.sync.dma_start(iota_t, iota_d[:, :])

            emb_t = scr.tile([VOCAB, D_HALF], dt.float32, tag="emb")
            nc.sync.dma_start(emb_t, emb_d[:, :])

            w1a = scr.tile([128, D_PAIR], dt.float32, tag="w1a")
            nc.sync.dma_start(w1a, wp_d[0:128, :])
            w1b = scr.tile([16, D_PAIR], dt.float32, tag="w1b")
            nc.sync.dma_start(w1b, wp_d[128:144, :])
            w2a = scr.tile([128, D_PAIR], dt.float32, tag="w2a")
            nc.sync.dma_start(w2a, wp_d[144:272, :])
            w2b = scr.tile([16, D_PAIR], dt.float32, tag="w2b")
            nc.sync.dma_start(w2b, wp_d[272:288, :])
            nc.sync.dma_start(wsep_t, _bcast(wp_d[288:289, :], 128))

            bp_t = scr.tile([1, D_PAIR], dt.float32, tag="bp")
            nc.sync.dma_start(bp_t, bp_d[:])

            aaB_i = scr.tile([128, L], dt.int32, tag="aaBi")
            nc.sync.dma_start(aaB_i, _bcast(aab_d[:], 128))
            seqB_i = scr.tile([VOCAB, L], dt.int32, tag="seqBi")
            nc.sync.dma_start(seqB_i, _bcast(seqb_d[:], VOCAB))
            seqI_i = scr.tile([VOCAB, IH], dt.int32, tag="seqIi")
            nc.sync.dma_start(seqI_i, _bcast(seqi_d[:], VOCAB))
            aaIB_i = scr.tile([128, IH], dt.int32, tag="aaIBi")
            nc.sync.dma_start(aaIB_i, _bcast(aai_d[:], 128))
            aaCol_i = scr.tile([IH, 1], dt.int32, tag="aaColi")
            nc.sync.dma_start(aaCol_i, aai_d[:])

            posL = scr.tile([128, 3 * D_PAIR], dt.float32, tag="posL")
            nc.sync.dma_start(posL, posL_d[:, :])
            posR = scr.tile([128, 3 * D_PAIR], dt.float32, tag="posR")
            nc.sync.dma_start(posR, posR_d[:, :])

            aaB_f = scr.tile([128, L], dt.float32, tag="aaBf")
            nc.vector.tensor_copy(aaB_f, aaB_i)
            seqB_f = scr.tile([VOCAB, L], dt.float32, tag="seqBf")
            nc.vector.tensor_copy(seqB_f, seqB_i)
            seqI_f = scr.tile([VOCAB, IH], dt.float32, tag="seqIf")
            nc.vector.tensor_copy(seqI_f, seqI_i)
            aaIB_f = scr.tile([128, IH], dt.float32, tag="aaIBf")
            nc.vector.tensor_copy(aaIB_f, aaIB_i)
            aaCol_f = scr.tile([IH, 1], dt.float32, tag="aaColf")
            nc.vector.tensor_copy(aaCol_f, aaCol_i)

            ohSeq = scr.tile([VOCAB, L], dt.float32, tag="ohSeq")
            nc.vector.tensor_scalar(
                ohSeq, seqB_f, iota_t[0:VOCAB, 0:1], None, ALU.is_equal
            )
            ohSeqI = scr.tile([VOCAB, IH], dt.float32, tag="ohSeqI")
            nc.vector.tensor_scalar(
                ohSeqI, seqI_f, iota_t[0:VOCAB, 0:1], None, ALU.is_equal
            )
            ohP = []
            ohPi = []
            for c in range(3):
                t = scr.tile([128, L], dt.float32, tag=f"ohP{c}", name=f"ohP{c}")
                nc.vector.tensor_scalar(t, aaB_f, iota_t[:, c : c + 1], None, ALU.is_equal)
                ohP.append(t)
                ti = scr.tile([128, IH], dt.float32, tag=f"ohPi{c}", name=f"ohPi{c}")
                nc.vector.tensor_scalar(
                    ti, aaIB_f, iota_t[:, c : c + 1], None, ALU.is_equal
                )
                ohPi.append(ti)

            seT_a_ps = psc.tile([128, L], dt.float32, tag="seTaP")
            nc.tensor.matmul(seT_a_ps, emb_t[:, 0:128], ohSeq, start=True, stop=True)
            seT_b_ps = psc.tile([16, L], dt.float32, tag="seTbP")
            nc.tensor.matmul(
                seT_b_ps, emb_t[:, 128:D_HALF], ohSeq, start=True, stop=True
            )
            seT_a = scr.tile([128, L], dt.float32, tag="seTa")
            nc.vector.tensor_copy(seT_a, seT_a_ps)
            seT_b = scr.tile([16, L], dt.float32, tag="seTb")
            nc.vector.tensor_copy(seT_b, seT_b_ps)

            seTi_a_ps = psc.tile([128, IH], dt.float32, tag="seTiaP")
            nc.tensor.matmul(
                seTi_a_ps, emb_t[:, 0:128], ohSeqI, start=True, stop=True
            )
            seTi_b_ps = psc.tile([16, IH], dt.float32, tag="seTibP")
            nc.tensor.matmul(
                seTi_b_ps, emb_t[:, 128:D_HALF], ohSeqI, start=True, stop=True
            )
            seTi_a = scr.tile([128, IH], dt.float32, tag="seTia")
            nc.vector.tensor_copy(seTi_a, seTi_a_ps)
            seTi_b = scr.tile([16, IH], dt.float32, tag="seTib")
            nc.vector.tensor_copy(seTi_b, seTi_b_ps)

            for h in range(2):
                co_ps = psc.tile(
                    [128, D_PAIR], dt.float32, tag=f"co{h}", name=f"co{h}"
                )
                sl = slice(h * 128, (h + 1) * 128)
                nc.tensor.matmul(co_ps, seT_a[:, sl], w1a, start=True, stop=False)
                nc.tensor.matmul(co_ps, seT_b[:, sl], w1b, start=False, stop=False)
                for c in range(3):
                    nc.tensor.matmul(
                        co_ps,
                        ohP[c][:, sl],
                        posR[:, c * D_PAIR : (c + 1) * D_PAIR],
                        start=False,
                        stop=(c == 2),
                    )
                co_hi = scr.tile(
                    [128, D_PAIR], dt.bfloat16, tag=f"cohi{h}", name=f"cohi{h}"
                )
                nc.vector.tensor_copy(co_hi, co_ps)
                co_lo = scr.tile(
                    [128, D_PAIR], dt.bfloat16, tag=f"colo{h}", name=f"colo{h}"
                )
                nc.vector.tensor_sub(co_lo, co_ps, co_hi)
                dst = slice(h * 128 * D_PAIR, (h * 128 + 128) * D_PAIR)
                nc.sync.dma_start(flat_t[0:1, dst], co_hi)
                nc.sync.dma_start(flat_t[1:2, dst], co_lo)

            ones_f = scr.tile([1, IH], dt.float32, tag="onesf")
            nc.vector.memset(ones_f, 1.0)
            cp_ps = psc.tile([128, D_PAIR], dt.float32, tag="cpP")
            nc.tensor.matmul(cp_ps, seTi_a, w2a, start=True, stop=False)
            nc.tensor.matmul(cp_ps, seTi_b, w2b, start=False, stop=False)
            for c in range(3):
                nc.tensor.matmul(
                    cp_ps,
                    ohPi[c],
                    posL[:, c * D_PAIR : (c + 1) * D_PAIR],
                    start=False,
                    stop=False,
                )
            nc.tensor.matmul(cp_ps, ones_f, bp_t, start=False, stop=True)
            nc.vector.tensor_copy(cp_t, cp_ps)

            dist_t = scr.tile([IH, L], dt.float32, tag="dist")
            nc.vector.tensor_scalar(dist_t, aaB_f, aaCol_f, None, ALU.subtract)
            abs_t = scr.tile([IH, L], dt.float32, tag="abs")
            nc.scalar.activation(abs_t, dist_t, AF.Abs)
            nc.scalar.activation(sep_t, abs_t, AF.Ln, bias=1.0)

        if stage == "setup":
            dbg = ctx.enter_context(tc.tile_pool(name="dbg", bufs=1))
            dbf = dbg.tile([IH, D_PAIR], dt.float32, tag="dbf")
            nc.vector.tensor_copy(dbf, cp_t)
            nc.sync.dma_start(out_d[:, 0:1, :], dbf)
            nc.vector.tensor_copy(dbf, wsep_t)
            nc.sync.dma_start(out_d[:, 1:2, :], dbf)
            return nc

        psj = ctx.enter_context(tc.tile_pool(name="psj", bufs=8, space="PSUM"))
        obp = ctx.enter_context(tc.tile_pool(name="obp", bufs=2))
        ngroups = int(stage[5:]) if stage.startswith("jloop") else L // JG
        if variant == "dmaonly":
            obs = []
            for k in range(2):
                t = obp.tile([IH, JG * D_PAIR], dt.float32, tag="ob", name="ob")
                nc.vector.memset(t, 0.5)
                obs.append(t)
            for g in range(ngroups * repeat):
                g = g % ngroups
                eng = nc.sync if g % 2 == 0 else nc.scalar
                eng.dma_start(out_d[:, g * JG : (g + 1) * JG, :], obs[g % 2])
            return nc
        for g in range(ngroups * repeat):
            g = g % ngroups
            ob = obp.tile([IH, JG * D_PAIR], dt.float32, tag="ob", name="ob")
            for jj in range(JG):
                j = g * JG + jj
                ps = psj.tile([IH, D_PAIR], dt.float32, tag="ps", name="ps")
                nc.tensor.matmul(
                    ps,
                    ones_t[0:2, :],
                    flat_t[0:2, j * D_PAIR : (j + 1) * D_PAIR],
                    start=True,
                    stop=True,
                )
                osl = ob[:, jj * D_PAIR : (jj + 1) * D_PAIR]
                if variant == "nostt":
                    nc.vector.tensor_copy(osl, ps)
                elif variant == "sttsb":
                    nc.vector.scalar_tensor_tensor(
                        osl, wsep_t, sep_t[:, j : j + 1], cp_t, ALU.mult, ALU.add
                    )
                else:
                    nc.vector.scalar_tensor_tensor(
                        osl, wsep_t, sep_t[:, j : j + 1], ps, ALU.mult, ALU.add
                    )
                if variant not in ("nopool", "nostt", "sttsb"):
                    nc.gpsimd.tensor_add(osl, osl, cp_t)
            if variant != "nodma":
                eng = nc.sync if g % 2 == 0 else nc.scalar
                eng.dma_start(out_d[:, g * JG : (g + 1) * JG, :], ob)

    return nc


_NC_CACHE = []


def make_in_maps(seq, aa_idx, emb_table, W_proj, b_proj):
    seq = np.asarray(seq, dtype=np.int32)
    aa_idx = np.asarray(aa_idx, dtype=np.int32)
    emb_table = np.ascontiguousarray(np.asarray(emb_table, dtype=np.float32))
    W_proj = np.ascontiguousarray(np.asarray(W_proj, dtype=np.float32))
    b_proj = np.ascontiguousarray(np.asarray(b_proj, dtype=np.float32))
    in_maps = []
    for c in range(N_CORES):
        b, ih = c // 2, c % 2
        in_maps.append(
            {
                "seqb": np.ascontiguousarray(seq[b]),
                "seqi": np.ascontiguousarray(seq[b, ih * IH : (ih + 1) * IH]),
                "aab": np.ascontiguousarray(aa_idx[b]),
                "aai": np.ascontiguousarray(aa_idx[b, ih * IH : (ih + 1) * IH]),
                "emb": emb_table,
                "wp": W_proj,
                "bp": b_proj,
            }
        )
    return in_maps


def gather_out(results) -> np.ndarray:
    out = np.empty((B, L, L, D_PAIR), dtype=np.float32)
    for c in range(N_CORES):
        b, ih = c // 2, c % 2
        out[b, ih * IH : (ih + 1) * IH] = np.asarray(results[c]["out"])
    return out


def kernel(seq, aa_idx, emb_table, W_proj, b_proj) -> np.ndarray:
    if not _NC_CACHE:
        nc = build()
        nc.finalize()
        _NC_CACHE.append(nc)
    nc = _NC_CACHE[0]
    in_maps = make_in_maps(seq, aa_idx, emb_table, W_proj, b_proj)
    res = run_bass_kernel_spmd(nc, in_maps, core_ids=list(range(N_CORES)))
    return gather_out(res.results)

